# revision 40
# baseline (speedup 1.0000x reference)
"""CrossPSDLoss Trainium2 kernel (half-frame fp8 DoubleRow version).

Math (from the reference):
  res = target - pred; both [1024, 16384] f32.
  cross rows i=0..15: row i = concat_b x[b, 1024*i : 1024*(i+1)]  (length 1048576)
  Welch per row: 511 frames of 4096 (stride 2048), periodic-hann*2 window,
  rFFT, power, sum over frames -> S[n].  Loss only uses rows 8..15 and
  frequency bins 21..499, and the /T factors cancel in the ratio:
     out = (2/480) * sum_{row=8..15} sum_{n=21..499} S_res[row,n]/S_tgt[row,n]

Half-frame trick: the scaled periodic hann is win[k] = 1 - cos(theta k)
(theta = 2pi/4096), and win[k+2048] = 1 + cos(theta k).  With the
unwindowed half-frame DFT R_h[n] = sum_{k<2048} x[2048h+k] e^{-i theta n k}:
     F_f[n] = (R_f[n] - C_f[n]) + (-1)^n (R_{f+1}[n] + C_{f+1}[n]),
     C_h[n] = (R_h[n-1] + R_h[n+1]) / 2.
Each sample enters exactly one GEMM per (trig, bin-chunk) — the 50% frame
overlap is never recomputed, halving the main GEMM work.

Sharding: one Welch row per NeuronCore (8 rows, 8 cores); host sums the 8
per-core [128, 4] per-bin ratio tiles.  Host computes res, scales by 1/4
(keeps R in fp8e4m3 range; the ratio is scale-invariant), casts to fp8,
and pre-transposes to XT[p, t, b] = X[b, 128t + p] so all DMAs are
contiguous.

Per-core pipeline, per (input, trig, chunk-of-122-bins-with-halo) group:
  1. PE: 8 fp8 DoubleRow matmuls (K=256 each, 0.5 cycles/row in the cost
     model) -> psR[nin, 512] = R at the chunk's halo bin range x 512 halves.
  2. DVE: copy psR -> Rb (fp8, SBUF).
  3. PE: ONE DoubleRow matmul with a hand-built overlapping moving AP
     ([nin, 2@stride1, 511]) and the stacked tridiagonal weights
     T1 = tridiag(-1/2, 1, -1/2), T2 = diag((-1)^n) tridiag(1/2, 1, 1/2):
     psF = T1 @ Rb[:, 0:511] + T2 @ Rb[:, 1:512]  (both halves of F in one
     instruction, accumulated in PSUM; verified bit-exact on HW).
  4. ACT: Square activation with accum over the 511 frames -> E column.

Schedule: tgt groups run first (so xr can arrive late), cos chunks run as
a diagonal pipeline (chunk c's x-piece p at step c+p) to absorb the
staggered DMA arrivals, tails are software-pipelined one group behind the
mains, and the last group (res,sin,c3) is frame-split 384+127 so only a
short bounce/tridiag/square chain trails the final matmul.  The dummy
matmul prewarm keeps the cost model's PE clock ramp hot through the DMA
lead-in.
"""

import os
import sys
from contextlib import ExitStack

import numpy as np
import ml_dtypes

for _p in ("/opt/trn_rl_repo", "/root/.axon_site/_ro/trn_rl_repo"):
    if os.path.isdir(_p) and _p not in sys.path:
        sys.path.insert(0, _p)

import concourse.bass as bass
import concourse.mybir as mybir
from concourse import bacc, tile
from concourse.bass_utils import run_bass_kernel_spmd

FP8 = ml_dtypes.float8_e4m3

NSEG = 511
NH = 512             # half-frames
NBINS = 479          # bins 21..499
B0S = [21, 141, 261, 381]
CHUNKS = [120, 120, 120, 119]
N_CORES = 8
ROW0 = 8
XSCALE = 0.25


def _overlap_ap(ap2d):
    """[P, n+1]-ish 2D slice -> [P, 2, n] AP, dim1 stride 1 (overlapping):
    element (p, r, f) reads column f + r."""
    y = ap2d.unsqueeze(1).copy()
    v = y.ap
    v.pop(1)
    v.insert(1, (1, 2))
    return y


def _build_nc() -> bass.Bass:
    nc = bacc.Bacc("TRN2", target_bir_lowering=False, debug=False,
                   num_devices=N_CORES)
    dt = mybir.dt
    DR = mybir.MatmulPerfMode.DoubleRow

    xr_d = nc.dram_tensor("xr", [128, 8, 1024], dt.float8e4,
                          kind="ExternalInput")
    xt_d = nc.dram_tensor("xt", [128, 8, 1024], dt.float8e4,
                          kind="ExternalInput")
    # half-DFT weights, chunk-major [chunk, p, m, 128]; cols j < nin are
    # bins b0-1+j (halo included), the rest zero padding (never read).
    wc_d = nc.dram_tensor("wc", [4, 128, 16, 128], dt.float8e4,
                          kind="ExternalInput")
    ws_d = nc.dram_tensor("ws", [4, 128, 16, 128], dt.float8e4,
                          kind="ExternalInput")
    t_d = nc.dram_tensor("tmat", [128, 2, 128], dt.float8e4,
                         kind="ExternalInput")
    out_d = nc.dram_tensor("out", [128, 4], dt.float32,
                           kind="ExternalOutput")

    with ExitStack() as ctx:
        tc = ctx.enter_context(tile.TileContext(nc))
        xpool = ctx.enter_context(tc.tile_pool(name="x", bufs=1))
        wpool = ctx.enter_context(tc.tile_pool(name="w", bufs=1))
        psR = ctx.enter_context(tc.tile_pool(name="psR", bufs=4,
                                             space="PSUM"))
        psF = ctx.enter_context(tc.tile_pool(name="psF", bufs=3,
                                             space="PSUM"))
        rbpool = ctx.enter_context(tc.tile_pool(name="rb", bufs=5))
        scpool = ctx.enter_context(tc.tile_pool(name="sc", bufs=2))
        stat = ctx.enter_context(tc.tile_pool(name="stat", bufs=1))

        xr_t = xpool.tile([128, 8, 1024], dt.float8e4, tag="xr")
        xt_t = xpool.tile([128, 8, 1024], dt.float8e4, tag="xt")
        t_t = wpool.tile([128, 2, 128], dt.float8e4, tag="tmat")
        w_sb = {}
        for trig in range(2):
            for c in range(4):
                w_sb[(trig, c)] = wpool.tile([128, 16, 128], dt.float8e4,
                                             name=f"w{trig}_{c}",
                                             tag=f"w{trig}_{c}")

        # prewarm scratch memsets go first so the dummy matmul stream can
        # start as early as possible.
        dum_w = stat.tile([128, 2, 128], dt.float8e4, tag="dum_w")
        dum_x = stat.tile([128, 520], dt.float8e4, tag="dum_x")
        nc.vector.memset(dum_x[:, :], 0.0)
        nc.vector.memset(dum_w[:, :, :], 0.0)

        # E accumulators, column c = chunk c.  Partitions with no real bin
        # keep their memset value: res-E 0.0 / tgt-E 1.0 makes their ratio
        # an exact 0/2 = 0, so no masking is needed before the reduction.
        E = {}
        for xi in range(2):
            for trig in range(2):
                t_ = stat.tile([128, 4], dt.float32, name=f"E{xi}_{trig}",
                               tag=f"E{xi}_{trig}")
                nc.vector.memset(t_[:, :], 0.0 if xi == 0 else 1.0)
                E[(xi, trig)] = t_

        # DMA order = PE need order (tgt groups run before res groups).
        # x goes in 4 pieces of 2 t-blocks each, interleaved with the weight
        # chunks, matching the diagonal matmul emission below.
        nc.sync.dma_start(w_sb[(0, 0)][:, :, :], wc_d[0])
        nc.sync.dma_start(xt_t[:, 0:2, :], xt_d[:, 0:2, :])
        nc.sync.dma_start(w_sb[(0, 1)][:, :, :], wc_d[1])
        nc.sync.dma_start(xt_t[:, 2:4, :], xt_d[:, 2:4, :])
        nc.sync.dma_start(w_sb[(0, 2)][:, :, :], wc_d[2])
        nc.sync.dma_start(xt_t[:, 4:6, :], xt_d[:, 4:6, :])
        nc.sync.dma_start(w_sb[(0, 3)][:, :, :], wc_d[3])
        nc.sync.dma_start(xt_t[:, 6:8, :], xt_d[:, 6:8, :])
        nc.sync.dma_start(t_t[:, :, :], t_d[:, :, :])
        for c in range(4):
            nc.sync.dma_start(w_sb[(1, c)][:, :, :], ws_d[c])
        for j in range(4):
            nc.sync.dma_start(xr_t[:, 2 * j:2 * j + 2, :],
                              xr_d[:, 2 * j:2 * j + 2, :])

        def emit_piece(g, ps, piece, hs=slice(0, NH), first=False,
                       last=False):
            """The two k-pair matmuls of x-piece `piece` (t-blocks 2p, 2p+1)
            for group g, over half-frames hs, into psum tile ps."""
            xi, trig, c = g
            xtile = xr_t if xi == 0 else xt_t
            nin = CHUNKS[c] + 2
            n0, n1 = 2 * hs.start, 2 * hs.stop - 1
            for n_, i in enumerate((piece, piece + 4)):
                m = 2 * i
                a, t = divmod(m, 8)
                nc.tensor.matmul(
                    ps[:nin, hs],
                    w_sb[(trig, c)][:, m:m + 2, 0:nin],
                    xtile[:, t:t + 2, a + n0:a + n1:2],
                    start=(first and n_ == 0),
                    stop=(last and n_ == 1),
                    perf_mode=DR,
                )

        def emit_main(g):
            ps = psR.tile([128, NH], dt.float32, tag="psR")
            for piece in range(4):
                emit_piece(g, ps, piece, first=(piece == 0),
                           last=(piece == 3))
            return ps

        def emit_bounce(g, ps, hs=slice(0, NH)):
            xi, trig, c = g
            nin = CHUNKS[c] + 2
            rb = rbpool.tile([128, NH], dt.float8e4, tag="rb")
            nc.vector.tensor_copy(rb[:nin, hs], ps[:nin, hs])
            return rb

        late_reduce = []

        def emit_tail(g, rb, fs=slice(0, NSEG), acc=None, pool_sq=False):
            """Tridiag DoubleRow matmul + Square-accumulate for F columns
            fs (reads rb columns fs.start .. fs.stop).  pool_sq=True squares
            on the idle GPSIMD engine instead (to thin the ACT convoy at the
            kernel tail) and defers the free-axis reduce to a point where it
            cannot block DVE's bounce stream."""
            xi, trig, c = g
            rows = CHUNKS[c]
            nin = rows + 2
            ncol = fs.stop - fs.start
            pf = psF.tile([128, NSEG], dt.float32, tag="psF")
            nc.tensor.matmul(
                pf[:rows, 0:ncol],
                t_t[0:nin, :, 0:rows],
                _overlap_ap(rb[0:nin, fs]),
                start=True, stop=True,
                perf_mode=DR,
            )
            if acc is None:
                acc = E[(xi, trig)][:rows, c:c + 1]
            if pool_sq:
                nc.gpsimd.tensor_mul(tmpP[:rows, 0:ncol], pf[:rows, 0:ncol],
                                     pf[:rows, 0:ncol])
                late_reduce.append((acc, rows, ncol))
                return
            tmp = scpool.tile([128, NSEG], dt.float32, tag="sq")
            nc.scalar.activation(
                out=tmp[:rows, 0:ncol],
                in_=pf[:rows, 0:ncol],
                func=mybir.ActivationFunctionType.Square,
                accum_out=acc,
            )

        SR = stat.tile([128, 4], dt.float32, tag="SR")
        ST = stat.tile([128, 4], dt.float32, tag="ST")
        REC = stat.tile([128, 4], dt.float32, tag="REC")
        RATIO = stat.tile([128, 4], dt.float32, tag="RATIO")
        EA = stat.tile([128, 1], dt.float32, tag="EA")
        EB = stat.tile([128, 1], dt.float32, tag="EB")
        P1 = stat.tile([128, 1], dt.float32, tag="P1")
        PR = stat.tile([128, 1], dt.float32, tag="PR")
        tmpP = stat.tile([128, NSEG], dt.float32, tag="tmpP")
        nc.vector.memset(tmpP[0:1, 0:1], 0.0)
        nc.vector.memset(EA[:, :], 0.0)
        nc.vector.memset(EB[:, :], 0.0)

        # prewarm: the cost model resets the PE clock ramp on any engine
        # gap, so a stream of dummy matmuls bridges the DMA lead-in and has
        # the PE at full clock when the real matmuls start.
        pd = psF.tile([128, NSEG], dt.float32, tag="psF")
        for _ in range(int(os.environ.get("KERNEL_DUMMIES", "4"))):
            nc.tensor.matmul(
                pd[:, :],
                dum_w[:, :, :],
                _overlap_ap(dum_x[:, 0:NSEG]),
                start=True, stop=True,
                perf_mode=DR,
            )

        pend = []

        def flush_one():
            if pend:
                emit_tail(*pend.pop(0))

        LASTG = (0, 1, 3)
        for xi in (1, 0):
            # cos: diagonal pipeline — chunk c runs piece p at step c+p, so
            # chunk completions (and their tails) stagger instead of
            # bunching, and the PE absorbs the staggered DMA arrivals.
            pstiles = {c: psR.tile([128, NH], dt.float32, name="psRp",
                                   tag="psR")
                       for c in range(4)}
            for step in range(7):
                for c in range(4):
                    p = step - c
                    if 0 <= p <= 3:
                        emit_piece((xi, 0, c), pstiles[c], p,
                                   first=(p == 0), last=(p == 3))
                if step >= 3:
                    c = step - 3
                    rb = emit_bounce((xi, 0, c), pstiles[c])
                    pend.append(((xi, 0, c), rb))
                if step != 3:
                    flush_one()
            if xi == 0:
                # tgt squares are all long done: the denominator reciprocals
                # come off the critical path here.
                nc.vector.tensor_add(ST[:, :], E[(1, 0)][:, :],
                                     E[(1, 1)][:, :])
                nc.vector.reciprocal(REC[:, :], ST[:, :])
            # sin: sequential groups, software-pipelined with pending tails
            for c in range(4):
                g = (xi, 1, c)
                if g == LASTG:
                    continue
                ps = emit_main(g)
                rb = emit_bounce(g, ps)
                flush_one()
                pend.append((g, rb))

        # last group (res, sin, c3): asymmetric frame-split (384 + 127)
        # shortens the serial tail.
        ps_a = psR.tile([128, NH], dt.float32, tag="psR")
        for piece in range(4):
            emit_piece(LASTG, ps_a, piece, hs=slice(0, 385),
                       first=(piece == 0), last=(piece == 3))
        rb_a = emit_bounce(LASTG, ps_a, hs=slice(0, 385))
        flush_one()
        pend.append((LASTG, rb_a, slice(0, 384), EA[:119, 0:1]))
        ps_b = psR.tile([128, NH], dt.float32, tag="psR")
        for piece in range(4):
            emit_piece(LASTG, ps_b, piece, hs=slice(384, NH),
                       first=(piece == 0), last=(piece == 3))
            if piece == 1:
                while pend:
                    flush_one()
        rb_b = emit_bounce(LASTG, ps_b, hs=slice(384, NH))
        # deferred pool-square reduce (DVE is clear of bounces from here on)
        for acc, rows_, ncol_ in late_reduce:
            nc.vector.tensor_reduce(acc, tmpP[:rows_, 0:ncol_],
                                    axis=mybir.AxisListType.X,
                                    op=mybir.AluOpType.add)
        # off the critical path: P1 = res_cos[c3] + first-half accumulation,
        # PR = P1 * rec (so the final combine is a single fused ACT op)
        nc.vector.tensor_add(P1[:, :], E[(0, 0)][:, 3:4], EA[:, :])
        nc.vector.tensor_mul(PR[:, :], P1[:, :], REC[:, 3:4])
        # ratios for columns 0..2
        for c in range(3):
            cs = slice(c, c + 1)
            nc.vector.tensor_add(SR[:, cs], E[(0, 0)][:, cs], E[(0, 1)][:, cs])
            nc.vector.tensor_mul(RATIO[:, cs], SR[:, cs], REC[:, cs])
        emit_tail(LASTG, rb_b, fs=slice(384, NSEG), acc=EB[:119, 0:1])

        # critical tail fused on ACT (no cross-engine hop after the square):
        # RATIO3 = relu(EB * rec3 + P1 * rec3); all terms are >= 0, so relu
        # is the identity -- it is just the one ACT func that takes both an
        # AP scale and an AP bias.
        nc.scalar.activation(
            out=RATIO[:, 3:4],
            in_=EB[:, :],
            func=mybir.ActivationFunctionType.Relu,
            bias=PR[:, 0:1],
            scale=REC[:, 3:4],
        )
        nc.sync.dma_start(out_d[:, :], RATIO[:, :])

    nc.compile()
    return nc


def _build_w():
    """Unwindowed half-frame DFT weights + tridiagonal combine matrices."""
    theta = 2.0 * np.pi / 4096.0
    k = np.arange(2048, dtype=np.float64)

    def pack(fn):
        out = np.zeros((4, 128, 16, 128), np.float64)
        for c in range(4):
            nin = CHUNKS[c] + 2
            bins = np.arange(B0S[c] - 1, B0S[c] - 1 + nin, dtype=np.float64)
            W = fn(theta * np.outer(k, bins))       # [2048, nin]
            out[c, :, :, 0:nin] = W.reshape(16, 128, nin).transpose(1, 0, 2)
        return out.astype(FP8)

    # T tile: t[p, 0, j] = T1[j, p], t[p, 1, j] = T2[j, p] (tridiagonal,
    # translation-invariant, so one tile serves all chunks; all b0 are odd
    # so the (-1)^n column signs are identical across chunks).
    t = np.zeros((128, 2, 128), np.float64)
    for j in range(121):
        s = -1.0 if (j % 2 == 0) else 1.0   # (-1)^(b0+j), b0 odd
        t[j, 0, j] = -0.5
        t[j + 1, 0, j] = 1.0
        t[j + 2, 0, j] = -0.5
        t[j, 1, j] = 0.5 * s
        t[j + 1, 1, j] = 1.0 * s
        t[j + 2, 1, j] = 0.5 * s
    return {
        "wc": pack(np.cos),
        "ws": pack(np.sin),
        "tmat": t.astype(FP8),
    }


_CACHE: dict = {}


def _get_prog():
    if "nc" not in _CACHE:
        _CACHE["nc"] = _build_nc()
    return _CACHE["nc"]


def _get_w():
    if "w" not in _CACHE:
        _CACHE["w"] = _build_w()
    return _CACHE["w"]


def kernel(pred: np.ndarray, target: np.ndarray, _trace: bool = False):
    nc = _get_prog()
    w = _get_w()
    pred = np.asarray(pred)
    target = np.asarray(target)
    # only Welch rows 8..15 contribute -> only the right half of the columns
    tgt_half = np.ascontiguousarray(target[:, 8192:]).astype(np.float32)
    res_half = tgt_half - pred[:, 8192:]
    res8 = (res_half * XSCALE).astype(FP8)
    tgt8 = (tgt_half * XSCALE).astype(FP8)
    in_maps = []
    for i in range(N_CORES):
        sl = slice(1024 * i, 1024 * (i + 1))
        in_maps.append({
            "xr": np.ascontiguousarray(
                res8[:, sl].reshape(1024, 8, 128).transpose(2, 1, 0)),
            "xt": np.ascontiguousarray(
                tgt8[:, sl].reshape(1024, 8, 128).transpose(2, 1, 0)),
            **w,
        })
    res = run_bass_kernel_spmd(nc, in_maps, list(range(N_CORES)), trace=_trace)
    total = sum(res.results[i]["out"].astype(np.float64).sum()
                for i in range(N_CORES))
    out = np.array(total * (2.0 / 480.0), dtype=np.float32)
    if _trace:
        return out, res
    return out


# revision 43
# speedup vs baseline: 1.0039x; 1.0039x over previous
"""CrossPSDLoss Trainium2 kernel (half-frame fp8 DoubleRow version).

Math (from the reference):
  res = target - pred; both [1024, 16384] f32.
  cross rows i=0..15: row i = concat_b x[b, 1024*i : 1024*(i+1)]  (length 1048576)
  Welch per row: 511 frames of 4096 (stride 2048), periodic-hann*2 window,
  rFFT, power, sum over frames -> S[n].  Loss only uses rows 8..15 and
  frequency bins 21..499, and the /T factors cancel in the ratio:
     out = (2/480) * sum_{row=8..15} sum_{n=21..499} S_res[row,n]/S_tgt[row,n]

Half-frame trick: the scaled periodic hann is win[k] = 1 - cos(theta k)
(theta = 2pi/4096), and win[k+2048] = 1 + cos(theta k).  With the
unwindowed half-frame DFT R_h[n] = sum_{k<2048} x[2048h+k] e^{-i theta n k}:
     F_f[n] = (R_f[n] - C_f[n]) + (-1)^n (R_{f+1}[n] + C_{f+1}[n]),
     C_h[n] = (R_h[n-1] + R_h[n+1]) / 2.
Each sample enters exactly one GEMM per (trig, bin-chunk) — the 50% frame
overlap is never recomputed, halving the main GEMM work.

Sharding: one Welch row per NeuronCore (8 rows, 8 cores); host sums the 8
per-core [128, 4] per-bin ratio tiles.  Host computes res, scales by 1/4
(keeps R in fp8e4m3 range; the ratio is scale-invariant), casts to fp8,
and pre-transposes to XT[p, t, b] = X[b, 128t + p] so all DMAs are
contiguous.

Per-core pipeline, per (input, trig, chunk-of-122-bins-with-halo) group:
  1. PE: 8 fp8 DoubleRow matmuls (K=256 each, 0.5 cycles/row in the cost
     model) -> psR[nin, 512] = R at the chunk's halo bin range x 512 halves.
  2. DVE: copy psR -> Rb (fp8, SBUF).
  3. PE: ONE DoubleRow matmul with a hand-built overlapping moving AP
     ([nin, 2@stride1, 511]) and the stacked tridiagonal weights
     T1 = tridiag(-1/2, 1, -1/2), T2 = diag((-1)^n) tridiag(1/2, 1, 1/2):
     psF = T1 @ Rb[:, 0:511] + T2 @ Rb[:, 1:512]  (both halves of F in one
     instruction, accumulated in PSUM; verified bit-exact on HW).
  4. ACT: Square activation with accum over the 511 frames -> E column.

Schedule: tgt groups run first (so xr can arrive late), cos chunks run as
a diagonal pipeline (chunk c's x-piece p at step c+p) to absorb the
staggered DMA arrivals, tails are software-pipelined one group behind the
mains, and the last group (res,sin,c3) is frame-split 384+127 so only a
short bounce/tridiag/square chain trails the final matmul.  The dummy
matmul prewarm keeps the cost model's PE clock ramp hot through the DMA
lead-in.
"""

import os
import sys
from contextlib import ExitStack

import numpy as np
import ml_dtypes

for _p in ("/opt/trn_rl_repo", "/root/.axon_site/_ro/trn_rl_repo"):
    if os.path.isdir(_p) and _p not in sys.path:
        sys.path.insert(0, _p)

import concourse.bass as bass
import concourse.mybir as mybir
from concourse import bacc, tile
from concourse.bass_utils import run_bass_kernel_spmd

FP8 = ml_dtypes.float8_e4m3

NSEG = 511
NH = 512             # half-frames
NBINS = 479          # bins 21..499
B0S = [21, 141, 261, 381]
CHUNKS = [120, 120, 120, 119]
N_CORES = 8
ROW0 = 8
XSCALE = 0.25


def _overlap_ap(ap2d):
    """[P, n+1]-ish 2D slice -> [P, 2, n] AP, dim1 stride 1 (overlapping):
    element (p, r, f) reads column f + r."""
    y = ap2d.unsqueeze(1).copy()
    v = y.ap
    v.pop(1)
    v.insert(1, (1, 2))
    return y


def _build_nc() -> bass.Bass:
    nc = bacc.Bacc("TRN2", target_bir_lowering=False, debug=False,
                   num_devices=N_CORES)
    dt = mybir.dt
    DR = mybir.MatmulPerfMode.DoubleRow

    xr_d = nc.dram_tensor("xr", [128, 8, 1024], dt.float8e4,
                          kind="ExternalInput")
    xt_d = nc.dram_tensor("xt", [128, 8, 1024], dt.float8e4,
                          kind="ExternalInput")
    # half-DFT weights, chunk-major [chunk, p, m, 128]; cols j < nin are
    # bins b0-1+j (halo included), the rest zero padding (never read).
    wc_d = nc.dram_tensor("wc", [4, 128, 16, 128], dt.float8e4,
                          kind="ExternalInput")
    ws_d = nc.dram_tensor("ws", [4, 128, 16, 128], dt.float8e4,
                          kind="ExternalInput")
    t_d = nc.dram_tensor("tmat", [128, 2, 128], dt.float8e4,
                         kind="ExternalInput")
    out_d = nc.dram_tensor("out", [128, 4], dt.float32,
                           kind="ExternalOutput")

    with ExitStack() as ctx:
        tc = ctx.enter_context(tile.TileContext(nc))
        xpool = ctx.enter_context(tc.tile_pool(name="x", bufs=1))
        wpool = ctx.enter_context(tc.tile_pool(name="w", bufs=1))
        psR = ctx.enter_context(tc.tile_pool(name="psR", bufs=4,
                                             space="PSUM"))
        psF = ctx.enter_context(tc.tile_pool(name="psF", bufs=3,
                                             space="PSUM"))
        psT = ctx.enter_context(tc.tile_pool(name="psT", bufs=1,
                                             space="PSUM"))
        rbpool = ctx.enter_context(tc.tile_pool(name="rb", bufs=5))
        scpool = ctx.enter_context(tc.tile_pool(name="sc", bufs=2))
        stat = ctx.enter_context(tc.tile_pool(name="stat", bufs=1))

        xr_t = xpool.tile([128, 8, 1024], dt.float8e4, tag="xr")
        xt_t = xpool.tile([128, 8, 1024], dt.float8e4, tag="xt")
        t_t = wpool.tile([128, 2, 128], dt.float8e4, tag="tmat")
        w_sb = {}
        for trig in range(2):
            for c in range(4):
                w_sb[(trig, c)] = wpool.tile([128, 16, 128], dt.float8e4,
                                             name=f"w{trig}_{c}",
                                             tag=f"w{trig}_{c}")

        # prewarm scratch memsets go first so the dummy matmul stream can
        # start as early as possible.
        dum_w = stat.tile([128, 2, 128], dt.float8e4, tag="dum_w")
        dum_x = stat.tile([128, 520], dt.float8e4, tag="dum_x")
        nc.vector.memset(dum_x[:, :], 0.0)
        nc.vector.memset(dum_w[:, :, :], 0.0)

        # E accumulators, column c = chunk c.  Partitions with no real bin
        # keep their memset value: res-E 0.0 / tgt-E 1.0 makes their ratio
        # an exact 0/2 = 0, so no masking is needed before the reduction.
        E = {}
        for xi in range(2):
            for trig in range(2):
                t_ = stat.tile([128, 4], dt.float32, name=f"E{xi}_{trig}",
                               tag=f"E{xi}_{trig}")
                nc.vector.memset(t_[:, :], 0.0 if xi == 0 else 1.0)
                E[(xi, trig)] = t_

        # DMA order = PE need order (tgt groups run before res groups).
        # x goes in 4 pieces of 2 t-blocks each, interleaved with the weight
        # chunks, matching the diagonal matmul emission below.
        nc.sync.dma_start(w_sb[(0, 0)][:, :, :], wc_d[0])
        nc.sync.dma_start(xt_t[:, 0:2, :], xt_d[:, 0:2, :])
        nc.sync.dma_start(w_sb[(0, 1)][:, :, :], wc_d[1])
        nc.sync.dma_start(xt_t[:, 2:4, :], xt_d[:, 2:4, :])
        nc.sync.dma_start(w_sb[(0, 2)][:, :, :], wc_d[2])
        nc.sync.dma_start(xt_t[:, 4:6, :], xt_d[:, 4:6, :])
        nc.sync.dma_start(w_sb[(0, 3)][:, :, :], wc_d[3])
        nc.sync.dma_start(xt_t[:, 6:8, :], xt_d[:, 6:8, :])
        nc.sync.dma_start(t_t[:, :, :], t_d[:, :, :])
        for c in range(4):
            nc.sync.dma_start(w_sb[(1, c)][:, :, :], ws_d[c])
        for j in range(4):
            nc.sync.dma_start(xr_t[:, 2 * j:2 * j + 2, :],
                              xr_d[:, 2 * j:2 * j + 2, :])

        def emit_piece(g, ps, piece, hs=slice(0, NH), first=False,
                       last=False):
            """The two k-pair matmuls of x-piece `piece` (t-blocks 2p, 2p+1)
            for group g, over half-frames hs, into psum tile ps."""
            xi, trig, c = g
            xtile = xr_t if xi == 0 else xt_t
            nin = CHUNKS[c] + 2
            n0, n1 = 2 * hs.start, 2 * hs.stop - 1
            for n_, i in enumerate((piece, piece + 4)):
                m = 2 * i
                a, t = divmod(m, 8)
                nc.tensor.matmul(
                    ps[:nin, hs],
                    w_sb[(trig, c)][:, m:m + 2, 0:nin],
                    xtile[:, t:t + 2, a + n0:a + n1:2],
                    start=(first and n_ == 0),
                    stop=(last and n_ == 1),
                    perf_mode=DR,
                )

        def emit_main(g):
            ps = psR.tile([128, NH], dt.float32, tag="psR")
            for piece in range(4):
                emit_piece(g, ps, piece, first=(piece == 0),
                           last=(piece == 3))
            return ps

        def emit_bounce(g, ps, hs=slice(0, NH)):
            xi, trig, c = g
            nin = CHUNKS[c] + 2
            rb = rbpool.tile([128, NH], dt.float8e4, tag="rb")
            nc.vector.tensor_copy(rb[:nin, hs], ps[:nin, hs])
            return rb

        sqT = psT.tile([128, NSEG], dt.float32, tag="sqT")

        late_reduce = []

        def emit_tail(g, rb, fs=slice(0, NSEG), acc=None, pool_sq=False):
            """Tridiag DoubleRow matmul + Square-accumulate for F columns
            fs (reads rb columns fs.start .. fs.stop).  pool_sq=True squares
            on the idle GPSIMD engine instead (to thin the ACT convoy at the
            kernel tail) and defers the free-axis reduce to a point where it
            cannot block DVE's bounce stream."""
            xi, trig, c = g
            rows = CHUNKS[c]
            nin = rows + 2
            ncol = fs.stop - fs.start
            pf = psF.tile([128, NSEG], dt.float32, tag="psF")
            nc.tensor.matmul(
                pf[:rows, 0:ncol],
                t_t[0:nin, :, 0:rows],
                _overlap_ap(rb[0:nin, fs]),
                start=True, stop=True,
                perf_mode=DR,
            )
            if acc is None:
                acc = E[(xi, trig)][:rows, c:c + 1]
            if pool_sq:
                nc.gpsimd.tensor_mul(tmpP[:rows, 0:ncol], pf[:rows, 0:ncol],
                                     pf[:rows, 0:ncol])
                late_reduce.append((acc, rows, ncol))
                return
            nc.scalar.activation(
                out=sqT[:rows, 0:ncol],
                in_=pf[:rows, 0:ncol],
                func=mybir.ActivationFunctionType.Square,
                accum_out=acc,
            )

        SR = stat.tile([128, 4], dt.float32, tag="SR")
        ST = stat.tile([128, 4], dt.float32, tag="ST")
        REC = stat.tile([128, 4], dt.float32, tag="REC")
        RATIO = stat.tile([128, 4], dt.float32, tag="RATIO")
        EA = stat.tile([128, 1], dt.float32, tag="EA")
        EB = stat.tile([128, 1], dt.float32, tag="EB")
        P1 = stat.tile([128, 1], dt.float32, tag="P1")
        PR = stat.tile([128, 1], dt.float32, tag="PR")
        tmpP = stat.tile([128, NSEG], dt.float32, tag="tmpP")
        nc.vector.memset(tmpP[0:1, 0:1], 0.0)
        nc.vector.memset(EA[:, :], 0.0)
        nc.vector.memset(EB[:, :], 0.0)

        # prewarm: the cost model resets the PE clock ramp on any engine
        # gap, so a stream of dummy matmuls bridges the DMA lead-in and has
        # the PE at full clock when the real matmuls start.
        pd = psF.tile([128, NSEG], dt.float32, tag="psF")
        for _ in range(int(os.environ.get("KERNEL_DUMMIES", "4"))):
            nc.tensor.matmul(
                pd[:, :],
                dum_w[:, :, :],
                _overlap_ap(dum_x[:, 0:NSEG]),
                start=True, stop=True,
                perf_mode=DR,
            )

        pend = []

        def flush_one():
            if pend:
                emit_tail(*pend.pop(0))

        LASTG = (0, 1, 3)
        for xi in (1, 0):
            # cos: diagonal pipeline — chunk c runs piece p at step c+p, so
            # chunk completions (and their tails) stagger instead of
            # bunching, and the PE absorbs the staggered DMA arrivals.
            pstiles = {c: psR.tile([128, NH], dt.float32, name="psRp",
                                   tag="psR")
                       for c in range(4)}
            for step in range(7):
                for c in range(4):
                    p = step - c
                    if 0 <= p <= 3:
                        emit_piece((xi, 0, c), pstiles[c], p,
                                   first=(p == 0), last=(p == 3))
                if step >= 3:
                    c = step - 3
                    rb = emit_bounce((xi, 0, c), pstiles[c])
                    pend.append(((xi, 0, c), rb))
                if step != 3:
                    flush_one()
            if xi == 0:
                # tgt squares are all long done: the denominator reciprocals
                # come off the critical path here.
                nc.vector.tensor_add(ST[:, :], E[(1, 0)][:, :],
                                     E[(1, 1)][:, :])
                nc.vector.reciprocal(REC[:, :], ST[:, :])
            # sin: sequential groups, software-pipelined with pending tails
            for c in range(4):
                g = (xi, 1, c)
                if g == LASTG:
                    continue
                ps = emit_main(g)
                rb = emit_bounce(g, ps)
                flush_one()
                pend.append((g, rb))

        # last group (res, sin, c3): asymmetric frame-split (384 + 127)
        # shortens the serial tail.
        ps_a = psR.tile([128, NH], dt.float32, tag="psR")
        for piece in range(4):
            emit_piece(LASTG, ps_a, piece, hs=slice(0, 385),
                       first=(piece == 0), last=(piece == 3))
        rb_a = emit_bounce(LASTG, ps_a, hs=slice(0, 385))
        flush_one()
        pend.append((LASTG, rb_a, slice(0, 384), EA[:119, 0:1]))
        ps_b = psR.tile([128, NH], dt.float32, tag="psR")
        for piece in range(4):
            emit_piece(LASTG, ps_b, piece, hs=slice(384, NH),
                       first=(piece == 0), last=(piece == 3))
            if piece == 1:
                while pend:
                    flush_one()
        rb_b = emit_bounce(LASTG, ps_b, hs=slice(384, NH))
        # deferred pool-square reduce (DVE is clear of bounces from here on)
        for acc, rows_, ncol_ in late_reduce:
            nc.vector.tensor_reduce(acc, tmpP[:rows_, 0:ncol_],
                                    axis=mybir.AxisListType.X,
                                    op=mybir.AluOpType.add)
        # off the critical path: P1 = res_cos[c3] + first-half accumulation,
        # PR = P1 * rec (so the final combine is a single fused ACT op)
        nc.vector.tensor_add(P1[:, :], E[(0, 0)][:, 3:4], EA[:, :])
        nc.vector.tensor_mul(PR[:, :], P1[:, :], REC[:, 3:4])
        # ratios for columns 0..2
        for c in range(3):
            cs = slice(c, c + 1)
            nc.vector.tensor_add(SR[:, cs], E[(0, 0)][:, cs], E[(0, 1)][:, cs])
            nc.vector.tensor_mul(RATIO[:, cs], SR[:, cs], REC[:, cs])
        emit_tail(LASTG, rb_b, fs=slice(384, NSEG), acc=EB[:119, 0:1])

        # critical tail fused on ACT (no cross-engine hop after the square):
        # RATIO3 = relu(EB * rec3 + P1 * rec3); all terms are >= 0, so relu
        # is the identity -- it is just the one ACT func that takes both an
        # AP scale and an AP bias.
        nc.scalar.activation(
            out=RATIO[:, 3:4],
            in_=EB[:, :],
            func=mybir.ActivationFunctionType.Relu,
            bias=PR[:, 0:1],
            scale=REC[:, 3:4],
        )
        nc.sync.dma_start(out_d[:, :], RATIO[:, :])

    nc.compile()
    return nc


def _build_w():
    """Unwindowed half-frame DFT weights + tridiagonal combine matrices."""
    theta = 2.0 * np.pi / 4096.0
    k = np.arange(2048, dtype=np.float64)

    def pack(fn):
        out = np.zeros((4, 128, 16, 128), np.float64)
        for c in range(4):
            nin = CHUNKS[c] + 2
            bins = np.arange(B0S[c] - 1, B0S[c] - 1 + nin, dtype=np.float64)
            W = fn(theta * np.outer(k, bins))       # [2048, nin]
            out[c, :, :, 0:nin] = W.reshape(16, 128, nin).transpose(1, 0, 2)
        return out.astype(FP8)

    # T tile: t[p, 0, j] = T1[j, p], t[p, 1, j] = T2[j, p] (tridiagonal,
    # translation-invariant, so one tile serves all chunks; all b0 are odd
    # so the (-1)^n column signs are identical across chunks).
    t = np.zeros((128, 2, 128), np.float64)
    for j in range(121):
        s = -1.0 if (j % 2 == 0) else 1.0   # (-1)^(b0+j), b0 odd
        t[j, 0, j] = -0.5
        t[j + 1, 0, j] = 1.0
        t[j + 2, 0, j] = -0.5
        t[j, 1, j] = 0.5 * s
        t[j + 1, 1, j] = 1.0 * s
        t[j + 2, 1, j] = 0.5 * s
    return {
        "wc": pack(np.cos),
        "ws": pack(np.sin),
        "tmat": t.astype(FP8),
    }


_CACHE: dict = {}


def _get_prog():
    if "nc" not in _CACHE:
        _CACHE["nc"] = _build_nc()
    return _CACHE["nc"]


def _get_w():
    if "w" not in _CACHE:
        _CACHE["w"] = _build_w()
    return _CACHE["w"]


def kernel(pred: np.ndarray, target: np.ndarray, _trace: bool = False):
    nc = _get_prog()
    w = _get_w()
    pred = np.asarray(pred)
    target = np.asarray(target)
    # only Welch rows 8..15 contribute -> only the right half of the columns
    tgt_half = np.ascontiguousarray(target[:, 8192:]).astype(np.float32)
    res_half = tgt_half - pred[:, 8192:]
    res8 = (res_half * XSCALE).astype(FP8)
    tgt8 = (tgt_half * XSCALE).astype(FP8)
    in_maps = []
    for i in range(N_CORES):
        sl = slice(1024 * i, 1024 * (i + 1))
        in_maps.append({
            "xr": np.ascontiguousarray(
                res8[:, sl].reshape(1024, 8, 128).transpose(2, 1, 0)),
            "xt": np.ascontiguousarray(
                tgt8[:, sl].reshape(1024, 8, 128).transpose(2, 1, 0)),
            **w,
        })
    res = run_bass_kernel_spmd(nc, in_maps, list(range(N_CORES)), trace=_trace)
    total = sum(res.results[i]["out"].astype(np.float64).sum()
                for i in range(N_CORES))
    out = np.array(total * (2.0 / 480.0), dtype=np.float32)
    if _trace:
        return out, res
    return out
